# revision 2
# baseline (speedup 1.0000x reference)
"""Multi-head attention kernel for 8 TRN2 NeuronCores.

Problem: B=4, S=2048, D=1024, H=16, DK=DV=64 multi-head attention with a
0/1 mask, f32 reference.

Sharding: 8 cores = 4 batches x 2 head-groups (8 heads each). Each core
computes, for its (batch, head-group): Q/K/V projections, masked softmax
attention, and a PARTIAL output projection (its heads' slice of Wo). The
two partials per batch are summed on the host (the all-reduce of the
tensor-parallel hint, done host-side since full inputs/outputs pass
through the host anyway).

Device compute in bf16 with f32 PSUM accumulation:
 - Activations are pre-transposed on host: xq/xk/xv = x[b].T  [D, S].
 - Q^T,K^T computed as [j, s] (j = head-local 64-dim blocks packed in
   pairs across 128 partitions); Q pre-scaled by 1/sqrt(DK).
 - Scores computed TRANSPOSED: S^T[t, s] = sum_j K^T[j,t] Q^T[j,s], so
   exp(S^T) directly feeds the attn@V matmul as the moving operand.
 - Softmax without max-subtraction (scores ~N(0,1); validated range).
   Mask applied multiplicatively after exp: P = exp(S^T) * maskT.
 - attn@V: lhsT = [V | ones] per head (65 cols) -> O^T rows 0..63 plus
   the softmax denominator (rowsum) in row 64, free on the PE.
 - Normalization: reciprocal of rowsums bounced through DRAM and
   broadcast across partitions by DMA; one elementwise multiply.
 - Output projection: packed head-pairs (k=128) with Wo slice.
"""

import numpy as np
import ml_dtypes
from contextlib import ExitStack

import concourse.bass as bass
import concourse.mybir as mybir
import concourse.tile as tile
from concourse import bacc
import concourse.bass_utils as bass_utils

P = 128
S = 2048          # sequence length
D = 1024          # model dim
HG = 8            # heads per core
DK = 64           # head dim
JW = HG * DK      # 512: packed projection width per core
DO = D // P       # 8 d-outer subtiles
NT = S // P       # 16 t-blocks
SC = 4            # s-chunks
SCW = S // SC     # 512 chunk width
NPAIR = HG // 2   # 4 head pairs
VW = DK + 1       # 65: V columns + ones column

bf16 = mybir.dt.bfloat16
f32 = mybir.dt.float32
AF = mybir.ActivationFunctionType
ALU = mybir.AluOpType


def _build():
    nc = bacc.Bacc("TRN2", target_bir_lowering=False, debug=False, num_devices=8)

    xq = nc.dram_tensor("xq", [D, S], bf16, kind="ExternalInput").ap()
    xk = nc.dram_tensor("xk", [D, S], bf16, kind="ExternalInput").ap()
    xv = nc.dram_tensor("xv", [D, S], bf16, kind="ExternalInput").ap()
    mt = nc.dram_tensor("mt", [S, S], bf16, kind="ExternalInput").ap()
    wq = nc.dram_tensor("wq", [D, JW], bf16, kind="ExternalInput").ap()
    wk = nc.dram_tensor("wk", [D, JW], bf16, kind="ExternalInput").ap()
    wv = nc.dram_tensor("wv", [D, JW], bf16, kind="ExternalInput").ap()
    wo = nc.dram_tensor("wo", [JW, D], bf16, kind="ExternalInput").ap()
    out = nc.dram_tensor("out", [S, D], f32, kind="ExternalOutput").ap()
    rscr = nc.dram_tensor("rscr", [SC, 2, NPAIR, SCW], bf16, kind="Internal").ap()

    with tile.TileContext(nc) as tc:
        with ExitStack() as ctx:
            consts = ctx.enter_context(tc.tile_pool(name="consts", bufs=1))
            stream = ctx.enter_context(tc.tile_pool(name="stream", bufs=6))
            mpool = ctx.enter_context(tc.tile_pool(name="mask", bufs=2))
            ppool = ctx.enter_context(tc.tile_pool(name="pp", bufs=4))
            epool = ctx.enter_context(tc.tile_pool(name="ep", bufs=3))
            rpool = ctx.enter_context(tc.tile_pool(name="rp", bufs=1))
            opool = ctx.enter_context(tc.tile_pool(name="op", bufs=3))
            psum = ctx.enter_context(tc.tile_pool(name="psum", bufs=8, space="PSUM"))

            # ---- weights ----
            wq_sb = consts.tile([P, DO, JW], bf16, tag="wq")
            nc.sync.dma_start(wq_sb[:], wq.rearrange("(o p) j -> p o j", p=P))
            wk_sb = consts.tile([P, DO, JW], bf16, tag="wk")
            nc.sync.dma_start(wk_sb[:], wk.rearrange("(o p) j -> p o j", p=P))
            wv_sb = consts.tile([P, DO, JW], bf16, tag="wv")
            nc.sync.dma_start(wv_sb[:], wv.rearrange("(o p) j -> p o j", p=P))
            wo_sb = consts.tile([P, JW // P, D], bf16, tag="wo")
            nc.sync.dma_start(wo_sb[:], wo.rearrange("(o p) d -> p o d", p=P))

            # ---- persistent activations ----
            QT = consts.tile([P, NPAIR, S], bf16, tag="qt")   # [64*(h%2)+j, h//2, s]
            KT = consts.tile([P, NPAIR, S], bf16, tag="kt")
            V = consts.tile([P, NT, HG * VW], bf16, tag="v")  # [t_in, t_out, 65h + (v|ones)]
            CT = consts.tile([P, NPAIR, S], bf16, tag="ct")   # concat^T, normalized in place

            for h in range(HG):
                nc.vector.memset(V[:, :, h * VW + DK : h * VW + DK + 1], 1.0)

            # ---- Q/K projections: dst[jo*128+m, s] = sum_d w[d, jo*128+m] x[d, s]
            for xin, wsb, dst, scale in ((xq, wq_sb, QT, 1.0 / 8.0), (xk, wk_sb, KT, 1.0)):
                for st in range(SC):
                    ps = [psum.tile([P, SCW], f32, tag="ps", name=f"ps{j}") for j in range(NPAIR)]
                    for do in range(DO):
                        xt = stream.tile([P, SCW], bf16, tag="xt")
                        nc.sync.dma_start(
                            xt[:], xin[do * P : (do + 1) * P, st * SCW : (st + 1) * SCW]
                        )
                        for jo in range(NPAIR):
                            nc.tensor.matmul(
                                ps[jo][:],
                                lhsT=wsb[:, do, jo * P : (jo + 1) * P],
                                rhs=xt[:],
                                start=(do == 0),
                                stop=(do == DO - 1),
                            )
                    for jo in range(NPAIR):
                        nc.scalar.activation(
                            dst[:, jo, st * SCW : (st + 1) * SCW], ps[jo][:],
                            AF.Copy, scale=scale,
                        )

            # ---- V projection: V[t, v] = sum_d x[d, t]^T w[d, v], strided into [V|ones] slots
            for tb in range(NT):
                pv = psum.tile([P, JW], f32, tag="ps")
                for do in range(DO):
                    xt = stream.tile([P, P], bf16, tag="xvt")
                    nc.sync.dma_start(
                        xt[:], xv[do * P : (do + 1) * P, tb * P : (tb + 1) * P]
                    )
                    nc.tensor.matmul(
                        pv[:], lhsT=xt[:], rhs=wv_sb[:, do, :],
                        start=(do == 0), stop=(do == DO - 1),
                    )
                nc.vector.tensor_copy(
                    V.rearrange("p t (h c) -> p t h c", h=HG)[:, tb, :, 0:DK],
                    pv.rearrange("p (h c) -> p h c", h=HG),
                )

            # ---- attention ----
            for sc in range(SC):
                mk = mpool.tile([P, NT, SCW], bf16, tag="mk")
                nc.sync.dma_start(
                    mk[:],
                    mt.rearrange("(to p) s -> p to s", p=P)[:, :, sc * SCW : (sc + 1) * SCW],
                )
                R = rpool.tile([P, HG, SCW], f32, tag="R")  # rowsums at partition 64

                for h in range(HG):
                    o = h // 2
                    base = 64 * (h % 2)
                    Ops = psum.tile([P, SCW], f32, tag="ps")
                    for tb in range(NT):
                        Sps = psum.tile([P, SCW], f32, tag="ps")
                        nc.tensor.matmul(
                            Sps[:],
                            lhsT=KT[base : base + DK, o, tb * P : (tb + 1) * P],
                            rhs=QT[base : base + DK, o, sc * SCW : (sc + 1) * SCW],
                            start=True, stop=True,
                        )
                        Pt = ppool.tile([P, SCW], bf16, tag="pt")
                        nc.scalar.activation(Pt[:], Sps[:], AF.Exp)
                        nc.vector.tensor_tensor(Pt[:], Pt[:], mk[:, tb, :], ALU.mult)
                        nc.tensor.matmul(
                            Ops[0:VW, :],
                            lhsT=V[:, tb, h * VW : (h + 1) * VW],
                            rhs=Pt[:],
                            start=(tb == 0), stop=(tb == NT - 1),
                        )
                    # evacuate rowsum + O^T
                    nc.vector.tensor_copy(R[64:65, h, :], Ops[64:65, :])
                    if h % 2 == 0:
                        nc.vector.tensor_copy(
                            CT[0:64, o, sc * SCW : (sc + 1) * SCW], Ops[0:64, :]
                        )
                    else:
                        ob = epool.tile([64, SCW], bf16, tag="ob")
                        nc.vector.tensor_copy(ob[:], Ops[0:64, :])
                        nc.sync.dma_start(
                            CT[64:128, o, sc * SCW : (sc + 1) * SCW], ob[:]
                        )

                # ---- normalization factors ----
                nc.vector.reciprocal(R[64:65, :, :], R[64:65, :, :])
                Rb = rpool.tile([P, HG, SCW], bf16, tag="Rb")
                nc.vector.tensor_copy(Rb[64:65, :, :], R[64:65, :, :])
                for h in range(HG):
                    nc.sync.dma_start(
                        rscr[sc, h % 2, h // 2 : h // 2 + 1, :], Rb[64:65, h, :]
                    )
                Rf = rpool.tile([P, NPAIR, SCW], bf16, tag="Rf")
                for par in range(2):
                    nc.sync.dma_start(
                        Rf[64 * par : 64 * par + 64, :, :],
                        rscr[sc, par].unsqueeze(0).to_broadcast([64, NPAIR, SCW]),
                    )
                nc.vector.tensor_tensor(
                    CT[:, :, sc * SCW : (sc + 1) * SCW],
                    CT[:, :, sc * SCW : (sc + 1) * SCW],
                    Rf[:], ALU.mult,
                )

                # ---- output projection for this chunk ----
                for sb in range(SCW // P):
                    s0 = sc * SCW + sb * P
                    for dt in range(D // SCW):
                        po = psum.tile([P, SCW], f32, tag="ps")
                        for o in range(NPAIR):
                            nc.tensor.matmul(
                                po[:],
                                lhsT=CT[:, o, s0 : s0 + P],
                                rhs=wo_sb[:, o, dt * SCW : (dt + 1) * SCW],
                                start=(o == 0), stop=(o == NPAIR - 1),
                            )
                        ot = opool.tile([P, SCW], f32, tag="ot")
                        nc.vector.tensor_copy(ot[:], po[:])
                        nc.sync.dma_start(
                            out[s0 : s0 + P, dt * SCW : (dt + 1) * SCW], ot[:]
                        )

    nc.compile()
    return nc


_NC = None


def _get_nc():
    global _NC
    if _NC is None:
        _NC = _build()
    return _NC


def kernel(queries, keys, values, mask, Wq, Wk, Wv, Wo):
    bf = ml_dtypes.bfloat16
    B = queries.shape[0]
    nc = _get_nc()

    xqT = [queries[b].T.astype(bf) for b in range(B)]
    xkT = [keys[b].T.astype(bf) for b in range(B)]
    xvT = [values[b].T.astype(bf) for b in range(B)]
    mtT = [(mask[b] != 0).T.astype(bf) for b in range(B)]
    wqg = [np.transpose(Wq[HG * g : HG * (g + 1)], (1, 0, 2)).reshape(D, JW).astype(bf)
           for g in range(2)]
    wkg = [np.transpose(Wk[HG * g : HG * (g + 1)], (1, 0, 2)).reshape(D, JW).astype(bf)
           for g in range(2)]
    wvg = [np.transpose(Wv[HG * g : HG * (g + 1)], (1, 0, 2)).reshape(D, JW).astype(bf)
           for g in range(2)]
    wog = [Wo[JW * g : JW * (g + 1), :].astype(bf) for g in range(2)]

    in_maps = []
    for c in range(8):
        b, g = c // 2, c % 2
        in_maps.append({
            "xq": xqT[b], "xk": xkT[b], "xv": xvT[b], "mt": mtT[b],
            "wq": wqg[g], "wk": wkg[g], "wv": wvg[g], "wo": wog[g],
        })

    res = bass_utils.run_bass_kernel_spmd(nc, in_maps, core_ids=list(range(8)))
    outs = [r["out"] for r in res.results]
    return np.stack([outs[2 * b] + outs[2 * b + 1] for b in range(B)]).astype(np.float32)


# revision 12
# speedup vs baseline: 1.3441x; 1.3441x over previous
"""Multi-head attention kernel for 8 TRN2 NeuronCores.

Problem: B=4, S=2048, D=1024, H=16, DK=DV=64 multi-head attention with a
0/1 mask, f32 reference.

Sharding: 8 cores = 4 batches x 2 head-groups (8 heads each). Each core
computes, for its (batch, head-group): Q/K/V projections, masked softmax
attention, and a PARTIAL output projection (its heads' slice of Wo). The
two partials per batch are summed on the host (the tensor-parallel
all-reduce of the sharding hint, done host-side since full inputs/outputs
pass through the host anyway).

Device compute in bf16 with f32 PSUM accumulation:
 - Activations are pre-transposed on host: xq/xk/xv = x[b].T  [D, S].
 - Q^T,K^T computed as [j, s] (head pairs packed across 128 partitions);
   Q pre-scaled by 1/sqrt(DK).
 - Scores computed TRANSPOSED: S^T[t, s] = sum_j K^T[j,t] Q^T[j,s], so
   exp(S^T) directly feeds the attn@V matmul as the moving operand.
 - Softmax without max-subtraction (scores ~N(0,1); validated range).
   Mask applied multiplicatively after exp: P = exp(S^T) * maskT.
 - attn@V: lhsT = [V | ones] per head (65 cols) -> O^T rows 0..63 plus
   the softmax denominator (rowsum) in row 64, free on the PE.
 - t-blocks processed in pairs through a 2-bank PSUM tile (3 buffers):
   one exp and one mask multiply per pair, keeping PE runs uniform and
   per-op overheads amortized.
 - Normalization: rowsums DMA'd to partitions 0..7, reciprocal there,
   bounced through DRAM and broadcast across partitions by DMA; one
   elementwise multiply on the packed concat^T.
 - Output projection: packed head-pairs (k=128), result DMA'd straight
   from PSUM to DRAM.
"""

import numpy as np
import ml_dtypes
from contextlib import ExitStack

import concourse.bass as bass
import concourse.mybir as mybir
import concourse.tile as tile
from concourse import bacc
import concourse.bass_utils as bass_utils

P = 128
S = 2048          # sequence length
D = 1024          # model dim
HG = 8            # heads per core
DK = 64           # head dim
JW = HG * DK      # 512: packed projection width per core
DO = D // P       # 8 d-outer subtiles
NT = S // P       # 16 t-blocks
SC = 4            # s-chunks
SCW = S // SC     # 512 chunk width
NPAIR = HG // 2   # 4 head pairs
VW = DK + 1       # 65: V columns + ones column

bf16 = mybir.dt.bfloat16
f32 = mybir.dt.float32
AF = mybir.ActivationFunctionType
ALU = mybir.AluOpType


def _build():
    nc = bacc.Bacc("TRN2", target_bir_lowering=False, debug=False, num_devices=8)

    xq = nc.dram_tensor("xq", [D, S], bf16, kind="ExternalInput").ap()
    xk = nc.dram_tensor("xk", [D, S], bf16, kind="ExternalInput").ap()
    xv = nc.dram_tensor("xv", [D, S], bf16, kind="ExternalInput").ap()
    mt = nc.dram_tensor("mt", [S, S], bf16, kind="ExternalInput").ap()
    wq = nc.dram_tensor("wq", [D, JW], bf16, kind="ExternalInput").ap()
    wk = nc.dram_tensor("wk", [D, JW], bf16, kind="ExternalInput").ap()
    wv = nc.dram_tensor("wv", [D, JW], bf16, kind="ExternalInput").ap()
    wo = nc.dram_tensor("wo", [JW, D], bf16, kind="ExternalInput").ap()
    out = nc.dram_tensor("out", [S, D], f32, kind="ExternalOutput").ap()
    rscr = nc.dram_tensor("rscr", [SC, HG, SCW], bf16, kind="Internal").ap()
    rsum = nc.dram_tensor("rsum", [SC, HG, SCW], f32, kind="Internal").ap()

    with tile.TileContext(nc) as tc:
        with ExitStack() as ctx:
            consts = ctx.enter_context(tc.tile_pool(name="consts", bufs=1))
            stream = ctx.enter_context(tc.tile_pool(name="stream", bufs=6))
            mpool = ctx.enter_context(tc.tile_pool(name="mask", bufs=2))
            ppool = ctx.enter_context(tc.tile_pool(name="pp", bufs=4))
            epool = ctx.enter_context(tc.tile_pool(name="ep", bufs=3))
            rpool = ctx.enter_context(tc.tile_pool(name="rp", bufs=1))
            psA = ctx.enter_context(tc.tile_pool(name="psA", bufs=3, space="PSUM"))
            psO = ctx.enter_context(tc.tile_pool(name="psO", bufs=2, space="PSUM"))

            # ---- weights ----
            wq_sb = consts.tile([P, DO, JW], bf16, tag="wq")
            nc.sync.dma_start(wq_sb[:], wq.rearrange("(o p) j -> p o j", p=P))
            wk_sb = consts.tile([P, DO, JW], bf16, tag="wk")
            nc.sync.dma_start(wk_sb[:], wk.rearrange("(o p) j -> p o j", p=P))
            wv_sb = consts.tile([P, DO, JW], bf16, tag="wv")
            nc.sync.dma_start(wv_sb[:], wv.rearrange("(o p) j -> p o j", p=P))
            wo_sb = consts.tile([P, JW // P, D], bf16, tag="wo")
            nc.sync.dma_start(wo_sb[:], wo.rearrange("(o p) d -> p o d", p=P))

            # ---- persistent activations ----
            QT = consts.tile([P, NPAIR, S], bf16, tag="qt")   # [64*(h%2)+j, h//2, s]
            KT = consts.tile([P, NPAIR, S], bf16, tag="kt")
            V = consts.tile([P, NT, HG * VW], bf16, tag="v")  # [t_in, t_out, 65h + (v|ones)]
            CT = consts.tile([P, NPAIR, S], bf16, tag="ct")   # concat^T, normalized in place

            for h in range(HG):
                nc.vector.memset(V[:, :, h * VW + DK : h * VW + DK + 1], 1.0)

            # ---- Q/K projections: dst[jo*128+m, s] = sum_d w[d, jo*128+m] x[d, s]
            for xin, wsb, dst, scale in ((xq, wq_sb, QT, 1.0 / 8.0), (xk, wk_sb, KT, 1.0)):
                for st in range(SC):
                    pa = psA.tile([P, 2, SCW], f32, tag="s2", name="pa")
                    pb = psA.tile([P, 2, SCW], f32, tag="s2", name="pb")
                    for do in range(DO):
                        xt = stream.tile([P, SCW], bf16, tag="xt")
                        nc.sync.dma_start(
                            xt[:], xin[do * P : (do + 1) * P, st * SCW : (st + 1) * SCW]
                        )
                        for jo in range(NPAIR):
                            tgt = pa if jo < 2 else pb
                            nc.tensor.matmul(
                                tgt[:, jo % 2, :],
                                lhsT=wsb[:, do, jo * P : (jo + 1) * P],
                                rhs=xt[:],
                                start=(do == 0),
                                stop=(do == DO - 1),
                            )
                    nc.scalar.activation(
                        dst[:, 0:2, st * SCW : (st + 1) * SCW], pa[:], AF.Copy, scale=scale
                    )
                    nc.scalar.activation(
                        dst[:, 2:4, st * SCW : (st + 1) * SCW], pb[:], AF.Copy, scale=scale
                    )

            # ---- V projection: V[t, v] = sum_d x[d, t]^T w[d, v], strided into [V|ones] slots
            for tb in range(NT):
                pv = psO.tile([P, SCW], f32, tag="o", name="pv")
                for do in range(DO):
                    xt = stream.tile([P, P], bf16, tag="xvt")
                    nc.sync.dma_start(
                        xt[:], xv[do * P : (do + 1) * P, tb * P : (tb + 1) * P]
                    )
                    nc.tensor.matmul(
                        pv[:], lhsT=xt[:], rhs=wv_sb[:, do, :],
                        start=(do == 0), stop=(do == DO - 1),
                    )
                nc.vector.tensor_copy(
                    V.rearrange("p t (h c) -> p t h c", h=HG)[:, tb, :, 0:DK],
                    pv.rearrange("p (h c) -> p h c", h=HG),
                )

            # ---- attention ----
            for sc in range(SC):
                mk = mpool.tile([P, NT, SCW], bf16, tag="mk")
                nc.sync.dma_start(
                    mk[:],
                    mt.rearrange("(to p) s -> p to s", p=P)[:, :, sc * SCW : (sc + 1) * SCW],
                )
                rs = rpool.tile([P, HG, SCW], f32, tag="rs")  # rowsum staging @ partition 64
                R2 = rpool.tile([P, SCW], f32, tag="R2")  # rowsums on partitions 0..7

                for h in range(HG):
                    o = h // 2
                    base = 64 * (h % 2)
                    Ops = psO.tile([P, SCW], f32, tag="o", name="Ops")
                    for g in range(NT // 2):
                        S2 = psA.tile([P, 2, SCW], f32, tag="s2", name="S2")
                        for i in range(2):
                            tb = 2 * g + i
                            nc.tensor.matmul(
                                S2[:, i, :],
                                lhsT=KT[base : base + DK, o, tb * P : (tb + 1) * P],
                                rhs=QT[base : base + DK, o, sc * SCW : (sc + 1) * SCW],
                                start=True, stop=True,
                            )
                        Pt = ppool.tile([P, 2, SCW], bf16, tag="pt")
                        nc.scalar.activation(Pt[:], S2[:], AF.Exp)
                        nc.vector.tensor_tensor(
                            Pt[:], Pt[:], mk[:, 2 * g : 2 * g + 2, :], ALU.mult
                        )
                        for i in range(2):
                            tb = 2 * g + i
                            nc.tensor.matmul(
                                Ops[0:VW, :],
                                lhsT=V[:, tb, h * VW : (h + 1) * VW],
                                rhs=Pt[:, i, :],
                                start=(tb == 0), stop=(tb == NT - 1),
                            )
                    # evacuate rowsum + O^T
                    nc.vector.tensor_copy(rs[64:65, h, :], Ops[64:65, :])
                    if h % 2 == 0:
                        nc.vector.tensor_copy(
                            CT[0:64, o, sc * SCW : (sc + 1) * SCW], Ops[0:64, :]
                        )
                    else:
                        ob = epool.tile([64, SCW], bf16, tag="ob")
                        nc.vector.tensor_copy(ob[:], Ops[0:64, :])
                        nc.sync.dma_start(
                            CT[64:128, o, sc * SCW : (sc + 1) * SCW], ob[:]
                        )

                # ---- normalization factors ----
                # bounce rowsums through DRAM to spread them over partitions 0..7
                nc.sync.dma_start(rsum[sc].unsqueeze(0), rs[64:65, :, :])
                nc.sync.dma_start(R2[0:HG, :], rsum[sc])
                Rr = rpool.tile([P, SCW], bf16, tag="Rr")
                nc.vector.reciprocal(R2[0:HG, :], R2[0:HG, :])
                nc.vector.tensor_copy(Rr[0:HG, :], R2[0:HG, :])
                nc.sync.dma_start(rscr[sc], Rr[0:HG, :])
                Rf = rpool.tile([P, NPAIR, SCW], bf16, tag="Rf")
                for par in range(2):
                    nc.sync.dma_start(
                        Rf[64 * par : 64 * par + 64, :, :],
                        rscr[sc].rearrange("(pr two) s -> two pr s", two=2)[par]
                        .unsqueeze(0).to_broadcast([64, NPAIR, SCW]),
                    )
                nc.vector.tensor_tensor(
                    CT[:, :, sc * SCW : (sc + 1) * SCW],
                    CT[:, :, sc * SCW : (sc + 1) * SCW],
                    Rf[:], ALU.mult,
                )

                # ---- output projection for this chunk (PSUM -> DRAM direct) ----
                for sb in range(SCW // P):
                    s0 = sc * SCW + sb * P
                    for dt in range(D // SCW):
                        po = psO.tile([P, SCW], f32, tag="o", name="po")
                        for o2 in range(NPAIR):
                            nc.tensor.matmul(
                                po[:],
                                lhsT=CT[:, o2, s0 : s0 + P],
                                rhs=wo_sb[:, o2, dt * SCW : (dt + 1) * SCW],
                                start=(o2 == 0), stop=(o2 == NPAIR - 1),
                            )
                        ot = epool.tile([P, SCW], f32, tag="ot")
                        nc.scalar.activation(ot[:], po[:], AF.Copy)
                        nc.sync.dma_start(
                            out[s0 : s0 + P, dt * SCW : (dt + 1) * SCW], ot[:]
                        )

    nc.compile()
    return nc


_NC = None


def _get_nc():
    global _NC
    if _NC is None:
        _NC = _build()
    return _NC


def kernel(queries, keys, values, mask, Wq, Wk, Wv, Wo):
    bf = ml_dtypes.bfloat16
    B = queries.shape[0]
    nc = _get_nc()

    xqT = [queries[b].T.astype(bf) for b in range(B)]
    xkT = [keys[b].T.astype(bf) for b in range(B)]
    xvT = [values[b].T.astype(bf) for b in range(B)]
    mtT = [(mask[b] != 0).T.astype(bf) for b in range(B)]
    wqg = [np.transpose(Wq[HG * g : HG * (g + 1)], (1, 0, 2)).reshape(D, JW).astype(bf)
           for g in range(2)]
    wkg = [np.transpose(Wk[HG * g : HG * (g + 1)], (1, 0, 2)).reshape(D, JW).astype(bf)
           for g in range(2)]
    wvg = [np.transpose(Wv[HG * g : HG * (g + 1)], (1, 0, 2)).reshape(D, JW).astype(bf)
           for g in range(2)]
    wog = [Wo[JW * g : JW * (g + 1), :].astype(bf) for g in range(2)]

    in_maps = []
    for c in range(8):
        b, g = c // 2, c % 2
        in_maps.append({
            "xq": xqT[b], "xk": xkT[b], "xv": xvT[b], "mt": mtT[b],
            "wq": wqg[g], "wk": wkg[g], "wv": wvg[g], "wo": wog[g],
        })

    res = bass_utils.run_bass_kernel_spmd(nc, in_maps, core_ids=list(range(8)))
    outs = [r["out"] for r in res.results]
    return np.stack([outs[2 * b] + outs[2 * b + 1] for b in range(B)]).astype(np.float32)
